# revision 21
# baseline (speedup 1.0000x reference)
"""AstrometryConcordanceHead Trainium2 kernel (8 NeuronCores, data parallel).

Sharding: core c -> batch b = c//2, image half (rows) = c%2.
Each core computes output rows [64h, 64h+64) of its batch element; no
cross-core communication. The host replicates edge rows into the shards so
the device program is SPMD-uniform; invalid (out-of-image) rows are zeroed
via per-row mask vectors baked into a consts tensor.

Feature-channel layout (partition index after the PE transpose):
  0 = dy, 32 = dx, 33..81 = corr (s = 7*(dy+3)+(dx+3)), 96 = conf, rest 0.
This keeps every partition-sliced access at a legal base (0/32/64/96).

Pipeline per core (bf16 matmuls, f32 elsewhere):
  - vis rows: L2-normalize (ACT square-accum) -> bf16 -> PE-transpose -> kn[d,x]
    134 wide with replicated x-edges
  - rubin rows: y-mix (DVE) -> x-upsample via a [64,128] interpolation matmul
    (scales fold into the cosine norm) -> normalize -> transpose -> qn[d,x]
  - correlation: 4 col-packed [32x38] all-pairs matmuls per (row, dy, d-chunk)
    -> PSUM [128, 266] -> SBUF -> DRAM scratch -> strided gather back as
    [x, 49] (band extraction: on-chip diagonal APs are illegal; DRAM APs not)
  - softmax over 49 shifts on [128x, 49] -> dy, dx, conf columns
  - PE-transpose [128,128] -> conv feature row
  - conv5x5 stack: per-ky matmuls, kx folded into M (col = 32*kx + oc),
    kx-collapse = ACT copies + DVE shifted adds, exact GELU on ACT
"""
import sys

sys.path.insert(0, "/opt/trn_rl_repo")

import numpy as np

F32_NP = np.float32

B, D = 4, 256
S2 = 49
NY = 76    # corr/feature rows per core (incl. up-to-6 invalid masked rows)
NK = 82    # k rows per core (edge-replicated by host)
NQ = 40    # rubin rows per core (edge-replicated by host)
NH1 = 72   # h1 rows (incl. masked pads)
NH2 = 68   # h2 rows
NOUT = 64
SKY = 1.6  # 2048 * 0.1 / 128

CH_DY, CH_DX, CH_CORR, CH_CONF = 0, 32, 33, 96

_CACHE = {}


def _host_consts(inputs):
    w0 = np.asarray(inputs["w0"], F32_NP)
    w1 = np.asarray(inputs["w1"], F32_NP)
    w2 = np.asarray(inputs["w2"], F32_NP)

    def pack(w, nin, ch_map):
        oc = w.shape[0]
        out = np.zeros((5, nin, 5 * oc), F32_NP)
        for ky in range(5):
            for kx in range(5):
                for ci, co in enumerate(ch_map):
                    out[ky, co, kx * oc:(kx + 1) * oc] = w[:, ci, ky, kx]
        return out

    # conv1 input channels in reference order [dy, dx, corr49, conf]
    ch_map1 = [CH_DY, CH_DX] + [CH_CORR + s for s in range(S2)] + [CH_CONF]
    w1p = pack(w0, 128, ch_map1)                      # [5, 128, 160]
    w2p = pack(w1, 32, list(range(32)))               # [5, 32, 160]
    w3p = pack(w2, 32, list(range(32)))               # [5, 32, 10]
    w3pp = np.zeros((5, 32, 160), F32_NP)
    for kx in range(5):
        w3pp[:, :, kx * 32:kx * 32 + 2] = w3p[:, :, kx * 2:kx * 2 + 2]

    cy = np.array([s // 7 - 3 for s in range(S2)], F32_NP)
    cx = np.array([s % 7 - 3 for s in range(S2)], F32_NP)
    ident = np.eye(128, dtype=F32_NP)
    u64 = np.zeros((64, 128), F32_NP)
    for i in range(64):
        xe = 2 * i
        u64[max(i - 1, 0), xe] += 1.0
        u64[i, xe] += 3.0
        xo = 2 * i + 1
        u64[i, xo] += 3.0
        u64[min(i + 1, 63), xo] += 1.0
    return w1p, w2p, w3pp, cy, cx, ident, u64


def _consts_blob(inputs, r0):
    w1p, w2p, w3pp, cy, cx, ident, u64 = _CACHE["host"]
    xm = ((np.arange(NY) + r0 - 6 >= 0) &
          (np.arange(NY) + r0 - 6 < 128)).astype(F32_NP)
    h1m = ((np.arange(NH1) + r0 - 4 >= 0) &
           (np.arange(NH1) + r0 - 4 < 128)).astype(F32_NP)
    h2m = ((np.arange(NH2) + r0 - 2 >= 0) &
           (np.arange(NH2) + r0 - 2 < 128)).astype(F32_NP)
    return np.concatenate([
        np.tile(cy, 128), np.tile(cx, 128),
        np.asarray(inputs["b0"], F32_NP), np.asarray(inputs["b1"], F32_NP),
        np.asarray(inputs["b2"], F32_NP),
        ident.reshape(-1), u64.reshape(-1),
        np.repeat(xm[None, :], 128, 0).reshape(-1),
        np.repeat(h1m[None, :], 32, 0).reshape(-1),
        np.repeat(h2m[None, :], 32, 0).reshape(-1)])


def _build(TEMP_INV):
    import concourse.bacc as bacc
    import concourse.mybir as mybir
    from concourse.ap import AP
    from concourse.tile import TileContext
    from contextlib import ExitStack

    F32 = mybir.dt.float32
    BF16 = mybir.dt.bfloat16
    AX = mybir.AxisListType
    ALU = mybir.AluOpType
    ACTF = mybir.ActivationFunctionType

    nc = bacc.Bacc("TRN2", target_bir_lowering=False, debug=False)

    rubin = nc.declare_dram_parameter("rubin", [NQ, 64, D], F32, isOutput=False)
    vis = nc.declare_dram_parameter("vis", [NK, 128, D], F32, isOutput=False)
    w1p_d = nc.declare_dram_parameter("w1p", [5, 128, 160], F32, isOutput=False)
    w2p_d = nc.declare_dram_parameter("w2p", [5, 32, 160], F32, isOutput=False)
    w3p_d = nc.declare_dram_parameter("w3p", [5, 32, 160], F32, isOutput=False)
    NCONST = (128 * 49) * 2 + 32 + 32 + 2 + 16384 + 64 * 128 \
        + 128 * NY + 32 * NH1 + 32 * NH2
    consts = nc.declare_dram_parameter("consts", [NCONST], F32, isOutput=False)
    out_ext = nc.declare_dram_parameter("out", [5, NOUT, 128], F32, isOutput=True)

    scratch = nc.dram_tensor("scratch", [NY, 128, 266], F32)

    OFF_CY = 0
    OFF_CX = OFF_CY + 128 * 49
    OFF_B0 = OFF_CX + 128 * 49
    OFF_B1 = OFF_B0 + 32
    OFF_B2 = OFF_B1 + 32
    OFF_ID = OFF_B2 + 2
    OFF_U = OFF_ID + 16384
    OFF_XM = OFF_U + 64 * 128
    OFF_H1M = OFF_XM + 128 * NY
    OFF_H2M = OFF_H1M + 32 * NH1

    with TileContext(nc) as tc, ExitStack() as es:
        cpool = es.enter_context(tc.tile_pool(name="consts", bufs=1))
        wpool = es.enter_context(tc.tile_pool(name="weights", bufs=1))
        persist = es.enter_context(tc.tile_pool(name="persist", bufs=1))
        ld = es.enter_context(tc.tile_pool(name="ld", bufs=4))
        q64p = es.enter_context(tc.tile_pool(name="q64", bufs=8))
        knp = es.enter_context(tc.tile_pool(name="kn", bufs=14))
        qnp = es.enter_context(tc.tile_pool(name="qn", bufs=4))
        work = es.enter_context(tc.tile_pool(name="work", bufs=4))
        small = es.enter_context(tc.tile_pool(name="small", bufs=6))
        cxp = es.enter_context(tc.tile_pool(name="corrx", bufs=3))
        ps_t = es.enter_context(tc.tile_pool(name="ps_t", bufs=2, space="PSUM"))
        ps_c = es.enter_context(tc.tile_pool(name="ps_c", bufs=2, space="PSUM"))
        ps_x = es.enter_context(tc.tile_pool(name="ps_x", bufs=1, space="PSUM"))
        ps_u = es.enter_context(tc.tile_pool(name="ps_u", bufs=1, space="PSUM"))
        ps_g1 = es.enter_context(tc.tile_pool(name="ps_g1", bufs=1, space="PSUM"))
        ps_g2 = es.enter_context(tc.tile_pool(name="ps_g2", bufs=1, space="PSUM"))

        # ---- constants ----
        def ldc(shape, off, n, p, tag):
            t = cpool.tile(shape, F32, tag=tag)
            nc.sync.dma_start(
                out=t[:],
                in_=consts.ap()[off:off + n].rearrange("(p n) -> p n", p=p))
            return t

        cyv = ldc([128, S2], OFF_CY, 128 * 49, 128, "cyv")
        cxv = ldc([128, S2], OFF_CX, 128 * 49, 128, "cxv")
        ident_f = ldc([128, 128], OFF_ID, 16384, 128, "identf")
        u64f = ldc([64, 128], OFF_U, 64 * 128, 64, "u64f")
        xmask = ldc([128, NY], OFF_XM, 128 * NY, 128, "xmask")
        h1mask = ldc([32, NH1], OFF_H1M, 32 * NH1, 32, "h1mask")
        h2mask = ldc([32, NH2], OFF_H2M, 32 * NH2, 32, "h2mask")
        b0t = cpool.tile([32, 1], F32)
        b1t = cpool.tile([32, 1], F32)
        b2t = cpool.tile([2, 1], F32)
        nc.sync.dma_start(out=b0t[:], in_=consts.ap()[OFF_B0:OFF_B0 + 32].unsqueeze(1))
        nc.sync.dma_start(out=b1t[:], in_=consts.ap()[OFF_B1:OFF_B1 + 32].unsqueeze(1))
        nc.sync.dma_start(out=b2t[:], in_=consts.ap()[OFF_B2:OFF_B2 + 2].unsqueeze(1))
        ident_b = cpool.tile([128, 128], BF16)
        nc.vector.tensor_copy(ident_b[:], ident_f[:])
        u64b = cpool.tile([64, 128], BF16)
        nc.vector.tensor_copy(u64b[:], u64f[:])

        w1sb, w2sb, w3sb = [], [], []
        for ky in range(5):
            for (lst, dram, npart, tag) in ((w1sb, w1p_d, 128, "w1"),
                                            (w2sb, w2p_d, 32, "w2"),
                                            (w3sb, w3p_d, 32, "w3")):
                tf = wpool.tile([npart, 160], F32, tag=f"{tag}f")
                nc.sync.dma_start(out=tf[:], in_=dram.ap()[ky])
                tb = wpool.tile([npart, 160], BF16, tag=f"{tag}_{ky}")
                nc.vector.tensor_copy(tb[:], tf[:])
                lst.append(tb)

        # ---- persistent feature storage ----
        x_all = persist.tile([128, NY, 132], BF16)
        h1_all = persist.tile([32, NH1, 132], BF16)
        h2_all = persist.tile([32, NH2, 132], BF16)
        nc.gpsimd.memset(x_all[:], 0.0)
        nc.gpsimd.memset(h1_all[:], 0.0)
        nc.gpsimd.memset(h2_all[:], 0.0)

        def normalize(pix_ap, tag):
            """pix_ap [128, 256] (SBUF or PSUM) -> bf16 normalized rows."""
            ss = small.tile([128, 1], F32, tag=f"ss{tag}")
            trash = work.tile([128, D], F32, tag="trash")
            nc.scalar.activation(trash[:], pix_ap, ACTF.Square, accum_out=ss[:])
            nrm = small.tile([128, 1], F32, tag=f"nrm{tag}")
            nc.scalar.activation(nrm[:], ss[:], ACTF.Sqrt)
            rinv = small.tile([128, 1], F32, tag=f"ri{tag}")
            nc.vector.reciprocal(rinv[:], nrm[:])
            pnorm = work.tile([128, D], BF16, tag=f"pn{tag}")
            nc.scalar.activation(pnorm[:], pix_ap, ACTF.Copy, scale=rinv[:])
            return pnorm

        kn_tiles = {}
        q64_tiles = {}

        def load_k(ki):
            kraw = ld.tile([128, D], F32, tag="kraw")
            nc.sync.dma_start(out=kraw[:], in_=vis.ap()[ki])
            pnorm = normalize(kraw[:], "k")
            kt = knp.tile([128, 2, 134], BF16, tag="kn")
            for ch in range(2):
                tp = ps_t.tile([128, 128], BF16, tag="tp")
                nc.tensor.transpose(tp[:], pnorm[:, ch * 128:(ch + 1) * 128],
                                    ident_b[:])
                nc.vector.tensor_copy(kt[:, ch, 3:131], tp[:])
                nc.vector.tensor_copy(kt[:, ch, 0:3],
                                      kt[:, ch, 3:4].to_broadcast([128, 3]))
                nc.vector.tensor_copy(kt[:, ch, 131:134],
                                      kt[:, ch, 130:131].to_broadcast([128, 3]))
            kn_tiles[ki] = kt

        def load_q64(j):
            qt = q64p.tile([64, D], F32, tag="q64")
            nc.sync.dma_start(out=qt[:], in_=rubin.ap()[j])
            q64_tiles[j] = qt

        for ki in range(7):
            load_k(ki)
        load_q64(0)
        load_q64(1)

        for t in range(NY):
            if t + 7 < NK:
                load_k(t + 7)
            if t % 2 == 0:
                ja, jb = t // 2, t // 2 + 1      # u = A + 3B
            else:
                ja, jb = t // 2 + 2, t // 2 + 1
            for j in (ja, jb):
                if j not in q64_tiles:
                    load_q64(j)
            u = work.tile([64, D], BF16, tag="u")
            nc.vector.scalar_tensor_tensor(u[:], q64_tiles[jb][:], 3.0,
                                           q64_tiles[ja][:],
                                           op0=ALU.mult, op1=ALU.add)
            # x-upsample: [128 fine x, 256 d] = u64b.T @ u
            qup = ps_u.tile([128, D], F32, tag="qup")
            nc.tensor.matmul(qup[:], lhsT=u64b[:], rhs=u[:],
                             start=True, stop=True)
            pnq = normalize(qup[:], "q")
            qn = qnp.tile([128, 2, 128], BF16, tag="qn")
            for ch in range(2):
                tp = ps_t.tile([128, 128], BF16, tag="tp")
                nc.tensor.transpose(tp[:], pnq[:, ch * 128:(ch + 1) * 128],
                                    ident_b[:])
                nc.vector.tensor_copy(qn[:, ch, :], tp[:])

            # correlation all-pairs
            cp = ps_c.tile([128, 266], F32, tag="cp")
            for dy in range(7):
                kt = kn_tiles[t + dy]
                for ch in range(2):
                    for b4 in range(4):
                        nc.tensor.matmul(
                            cp[32 * b4:32 * b4 + 32, 38 * dy:38 * dy + 38],
                            lhsT=qn[:, ch, 32 * b4:32 * b4 + 32],
                            rhs=kt[:, ch, 32 * b4:32 * b4 + 38],
                            start=(ch == 0), stop=(ch == 1),
                            tile_position=(0, 32 * b4))
            cps = work.tile([128, 266], F32, tag="cps")
            nc.vector.tensor_copy(cps[:], cp[:])
            nc.sync.dma_start(out=scratch.ap()[t], in_=cps[:])

            # band gather: corrX[x, CH_CORR + 7*dy + dx]
            corrX = cxp.tile([128, 128], F32, tag="corrX")
            nc.gpsimd.memset(corrX[:], 0.0)
            for b4 in range(4):
                src = AP(tensor=scratch, offset=t * 128 * 266 + 8512 * b4,
                         ap=[[267, 32], [38, 7], [1, 7]])
                dst = AP(tensor=corrX.tensor,
                         offset=corrX[:].offset + 128 * 32 * b4 + CH_CORR,
                         ap=[[128, 32], [7, 7], [1, 7]])
                nc.sync.dma_start(out=dst, in_=src)

            # softmax over the 49 shifts
            cslice = corrX[:, CH_CORR:CH_CORR + S2]
            mx = small.tile([128, 1], F32, tag="mx")
            nc.vector.reduce_max(out=mx[:], in_=cslice, axis=AX.X)
            ebias = small.tile([128, 1], F32, tag="eb")
            nc.vector.tensor_scalar_mul(ebias[:], mx[:], -TEMP_INV)
            e = work.tile([128, S2], F32, tag="e")
            ssum = small.tile([128, 1], F32, tag="ssum")
            nc.scalar.activation(e[:], cslice, ACTF.Exp,
                                 bias=ebias[:], scale=TEMP_INV, accum_out=ssum[:])
            rs = small.tile([128, 1], F32, tag="rs")
            nc.vector.reciprocal(rs[:], ssum[:])
            emax = small.tile([128, 1], F32, tag="emax")
            nc.vector.reduce_max(out=emax[:], in_=e[:], axis=AX.X)
            nc.vector.tensor_tensor(out=corrX[:, CH_CONF:CH_CONF + 1],
                                    in0=emax[:], in1=rs[:], op=ALU.mult)
            etr = work.tile([128, S2], F32, tag="etr")
            dyr = small.tile([128, 1], F32, tag="dyr")
            nc.vector.scalar_tensor_tensor(etr[:], e[:], 1.0, cyv[:],
                                           op0=ALU.mult, op1=ALU.mult,
                                           accum_out=dyr[:])
            nc.vector.tensor_tensor(out=corrX[:, CH_DY:CH_DY + 1],
                                    in0=dyr[:], in1=rs[:], op=ALU.mult)
            dxr = small.tile([128, 1], F32, tag="dxr")
            nc.vector.scalar_tensor_tensor(etr[:], e[:], 1.0, cxv[:],
                                           op0=ALU.mult, op1=ALU.mult,
                                           accum_out=dxr[:])
            nc.vector.tensor_tensor(out=corrX[:, CH_DX:CH_DX + 1],
                                    in0=dxr[:], in1=rs[:], op=ALU.mult)

            # zero invalid rows entirely, keeping conv pads exact zeros
            nc.vector.tensor_scalar_mul(corrX[:], corrX[:], xmask[:, t:t + 1])

            # transpose to [channel, x] and store
            tpx = ps_x.tile([128, 128], F32, tag="tpx")
            nc.tensor.transpose(tpx[:], corrX[:], ident_f[:])
            nc.scalar.activation(x_all[:, t, 2:130], tpx[:], ACTF.Copy)

        # ---- conv helpers ----
        def kx_collapse(g1, g2, nparts, tag):
            """sum_kx G[32kx : 32kx+nparts, kx : kx+128]; returns [nparts,128]."""
            c1 = work.tile([nparts, 128], F32, tag=f"c1{tag}")
            c2 = work.tile([nparts, 128], F32, tag=f"c2{tag}")
            c3 = work.tile([nparts, 128], F32, tag=f"c3{tag}")
            nc.scalar.activation(c1[:], g1[32:32 + nparts, 1:129], ACTF.Copy)
            nc.scalar.activation(c2[:], g1[64:64 + nparts, 2:130], ACTF.Copy)
            nc.scalar.activation(c3[:], g1[96:96 + nparts, 3:131], ACTF.Copy)
            a1 = work.tile([nparts, 128], F32, tag=f"a1{tag}")
            nc.vector.tensor_tensor(out=a1[:], in0=g1[0:nparts, 0:128],
                                    in1=c1[:], op=ALU.add)
            nc.vector.tensor_tensor(out=c2[:], in0=c2[:], in1=c3[:], op=ALU.add)
            nc.vector.tensor_tensor(out=a1[:], in0=a1[:], in1=c2[:], op=ALU.add)
            nc.vector.tensor_tensor(out=a1[:], in0=a1[:],
                                    in1=g2[0:nparts, 4:132], op=ALU.add)
            return a1

        # ---- conv1 ----
        for t1 in range(NH1):
            g1 = ps_g1.tile([128, 132], F32, tag="g1")
            g2 = ps_g2.tile([32, 132], F32, tag="g2")
            for ky in range(5):
                rhs = x_all[:, t1 + ky, :]
                nc.tensor.matmul(g1[:], lhsT=w1sb[ky][:, 0:128], rhs=rhs,
                                 start=(ky == 0), stop=(ky == 4))
                nc.tensor.matmul(g2[:], lhsT=w1sb[ky][:, 128:160], rhs=rhs,
                                 start=(ky == 0), stop=(ky == 4))
            a1 = kx_collapse(g1, g2, 32, "v1")
            gl = work.tile([32, 128], F32, tag="gl1")
            nc.scalar.activation(gl[:], a1[:], ACTF.Gelu, bias=b0t[:])
            nc.vector.tensor_scalar_mul(h1_all[:, t1, 2:130], gl[:],
                                        h1mask[:, t1:t1 + 1])

        # ---- conv2 ----
        for t2 in range(NH2):
            g1 = ps_g1.tile([128, 132], F32, tag="g1")
            g2 = ps_g2.tile([32, 132], F32, tag="g2")
            for ky in range(5):
                rhs = h1_all[:, t2 + ky, :]
                nc.tensor.matmul(g1[:], lhsT=w2sb[ky][:, 0:128], rhs=rhs,
                                 start=(ky == 0), stop=(ky == 4))
                nc.tensor.matmul(g2[:], lhsT=w2sb[ky][:, 128:160], rhs=rhs,
                                 start=(ky == 0), stop=(ky == 4))
            a1 = kx_collapse(g1, g2, 32, "v2")
            gl = work.tile([32, 128], F32, tag="gl2")
            nc.scalar.activation(gl[:], a1[:], ACTF.Gelu, bias=b1t[:])
            nc.vector.tensor_scalar_mul(h2_all[:, t2, 2:130], gl[:],
                                        h2mask[:, t2:t2 + 1])

        # ---- conv3 + output ----
        for t3 in range(NOUT):
            g1 = ps_g1.tile([128, 132], F32, tag="g1")
            g2 = ps_g2.tile([32, 132], F32, tag="g2")
            for ky in range(5):
                rhs = h2_all[:, t3 + ky, :]
                nc.tensor.matmul(g1[:], lhsT=w3sb[ky][:, 0:128], rhs=rhs,
                                 start=(ky == 0), stop=(ky == 4))
                nc.tensor.matmul(g2[:], lhsT=w3sb[ky][:, 128:160], rhs=rhs,
                                 start=(ky == 0), stop=(ky == 4))
            r1 = kx_collapse(g1, g2, 2, "v3")
            nc.vector.tensor_scalar(out=r1[:], in0=r1[:], scalar1=b2t[:],
                                    scalar2=None, op0=ALU.add)
            # res ch1 -> partition 0 (DMA has no partition-base restriction)
            res1 = work.tile([1, 128], F32, tag="res1")
            nc.sync.dma_start(out=res1[:], in_=r1[1:2, :])
            tx = t3 + 6
            rdy = work.tile([1, 128], F32, tag="rdy")
            rdx = work.tile([1, 128], F32, tag="rdx")
            rcf = work.tile([1, 128], F32, tag="rcf")
            nc.scalar.activation(rdy[:], x_all[CH_DY:CH_DY + 1, tx, 2:130],
                                 ACTF.Copy)
            nc.scalar.activation(rdx[:], x_all[CH_DX:CH_DX + 1, tx, 2:130],
                                 ACTF.Copy)
            nc.scalar.activation(rcf[:], x_all[CH_CONF:CH_CONF + 1, tx, 2:130],
                                 ACTF.Copy)
            ddec = work.tile([1, 128], F32, tag="ddec")
            nc.vector.tensor_tensor(out=ddec[:], in0=r1[0:1, :],
                                    in1=rdy[:], op=ALU.add)
            nc.vector.tensor_scalar_mul(ddec[:], ddec[:], SKY)
            dra = work.tile([1, 128], F32, tag="dra")
            nc.vector.tensor_tensor(out=dra[:], in0=res1[:],
                                    in1=rdx[:], op=ALU.add)
            nc.vector.tensor_scalar_mul(dra[:], dra[:], SKY)
            nc.sync.dma_start(out=out_ext.ap()[0, t3, :].unsqueeze(0), in_=dra[:])
            nc.sync.dma_start(out=out_ext.ap()[1, t3, :].unsqueeze(0), in_=ddec[:])
            nc.sync.dma_start(out=out_ext.ap()[2, t3, :].unsqueeze(0), in_=rdy[:])
            nc.sync.dma_start(out=out_ext.ap()[3, t3, :].unsqueeze(0), in_=rdx[:])
            nc.sync.dma_start(out=out_ext.ap()[4, t3, :].unsqueeze(0), in_=rcf[:])

    nc.compile()
    return nc


def _shards(inputs):
    rubin_t = np.asarray(inputs["rubin_tokens"], F32_NP).reshape(B, 64, 64, D)
    vis_t = np.asarray(inputs["vis_tokens"], F32_NP).reshape(B, 128, 128, D)
    w1p, w2p, w3pp = _CACHE["host"][0:3]

    in_maps = []
    for c in range(8):
        b = c // 2
        r0 = 64 * (c % 2)
        krows = np.clip(np.arange(r0 - 9, r0 + 73), 0, 127)
        qrows = np.clip(np.arange(r0 // 2 - 4, r0 // 2 + 36), 0, 63)
        in_maps.append({
            "rubin": np.ascontiguousarray(rubin_t[b, qrows]),
            "vis": np.ascontiguousarray(vis_t[b, krows]),
            "w1p": w1p, "w2p": w2p, "w3p": w3pp,
            "consts": _consts_blob(inputs, r0),
        })
    return in_maps


def kernel(**inputs):
    from concourse.bass_utils import run_bass_kernel_spmd

    _CACHE["host"] = _host_consts(inputs)
    if "nc" not in _CACHE:
        _CACHE["nc"] = _build(float(1.0 / np.exp(np.asarray(inputs["log_temp"],
                                                            F32_NP))))
    in_maps = _shards(inputs)
    res = run_bass_kernel_spmd(_CACHE["nc"], in_maps, core_ids=list(range(8)))
    out = np.zeros((B, 5, 128, 128), F32_NP)
    for c in range(8):
        b = c // 2
        r0 = 64 * (c % 2)
        out[b, :, r0:r0 + 64, :] = res.results[c]["out"]
    return out
